# revision 31
# baseline (speedup 1.0000x reference)
"""Masked multi-head attention (B=2, S=2048, E=1024, H=16, D=64) on 8 TRN2 cores.

Sharding: each core owns 2 heads (of 16) for BOTH batches.
  - QKV projections computed per-core for its 2 heads (transposed layouts);
    batch-1 projection blocks are interleaved between batch-0 attention units
    so the TensorEngine stays busy while ScalarE runs exp.
  - Attention: flash-style with transposed scores (scoresT[k, q] tiles),
    software-pipelined at k-tile granularity: the score matmuls of k-tile
    i+1 are emitted BEFORE the PV matmuls of k-tile i, so the PE streams
    the next scores while ScalarE runs exp on the previous tile (PSUM score
    tiles are hl-paired [128, 2, 512] so one pool buf covers both heads).
    Unsafe softmax (no max subtraction -- scores are ~N(0,1)); denominator
    accumulated via a leading ones-column prepended to V in the PV matmul;
    causal mask applied only on the [128,128] diagonal square, PV matmuls
    skip fully-masked leading q-columns of diagonal k-tiles.
  - Exchange head-parallel -> slot-parallel via TWO AllToAlls: slots 0-3
    (batch 0) exchanged right after batch-0 attention, hidden under batch-1
    compute; slots 4-7 at the end (only this one is exposed). AllToAll
    moves 8x less data than per-slot AllGathers and avoids CC serialization.
  - Output projection row-parallel over the received heads (each core picks
    its A2A output half via a pid//4-offset DMA), bias fused via ScalarE;
    each core emits a transposed [1024, 512] bf16 slice; host transposes,
    stacks and upcasts.

Compute dtype bf16 (fp32 PSUM accumulation). Exp is trimmed to the valid
q-range of diagonal k-tiles (ScalarE is the near-critical engine during
attention). HW exec time ~230us (from 256.6us baseline); rel-l2 error vs
the fp32 reference ~5.3e-3.
"""

import numpy as np
import ml_dtypes

BF16 = ml_dtypes.bfloat16

B, S, E, H, D = 2, 2048, 1024, 16, 64
P = 128
SG = B * S          # 4096 global sequence length (batch-major)
NKO = E // P        # 8 contraction tiles over E
NST = SG // P       # 32 seq tiles of 128
NSB = SG // 512     # 8 seq blocks of 512
QB = S // 512       # 4 q-blocks per batch

_built = None
LAST_RESULTS = None


def _build():
    global _built
    if _built is not None:
        return _built

    import concourse.bacc as bacc
    import concourse.mybir as mybir
    import concourse.tile as tile
    from concourse.bass import ds as bass_ds

    f32 = mybir.dt.float32
    bf16 = mybir.dt.bfloat16
    Exp = mybir.ActivationFunctionType.Exp
    Identity = mybir.ActivationFunctionType.Identity

    nc = bacc.Bacc("TRN2", target_bir_lowering=False, debug=False, num_devices=8)

    # x pre-rearranged on host to [partition, seq-block, ko, 512] so each
    # chunk DMA moves 8KB-contiguous lines per partition on both sides
    xT = nc.declare_dram_parameter("xT", [P, NSB, NKO, 512], bf16, isOutput=False)
    wq = nc.declare_dram_parameter("wq", [E, P], bf16, isOutput=False)
    wk = nc.declare_dram_parameter("wk", [E, P], bf16, isOutput=False)
    wv = nc.declare_dram_parameter("wv", [E, P], bf16, isOutput=False)
    wo = nc.declare_dram_parameter("wo", [E, E], bf16, isOutput=False)
    bo = nc.declare_dram_parameter("bo", [P, NKO], f32, isOutput=False)
    masks = nc.declare_dram_parameter("masks", [P, 2048], bf16, isOutput=False)
    outT = nc.declare_dram_parameter("outT", [E, 512], bf16, isOutput=True)

    # AllToAll exchange, split by batch so the first one hides under compute:
    #   a2a_in0 chunk j (j<4) = my heads' attention for slot j   (batch 0)
    #   a2a_in1 chunk j (j>=4) = my heads' attention for slot j  (batch 1)
    # (unwritten chunks carry junk to cores that ignore them)
    # a2a_out[g] = output of A2A g; core c reads half g = c//4: chunk r there
    # holds rank r's heads for core c's own slot.
    a2a_in0 = nc.dram_tensor("a2a_in0", [8, P, 512], bf16)
    a2a_in1 = nc.dram_tensor("a2a_in1", [8, P, 512], bf16)
    a2a_out = nc.dram_tensor("a2a_out", [2, 8, P, 512], bf16)

    with tile.TileContext(nc) as tc, \
         tc.tile_pool(name="const", bufs=1) as const:
        # ---- constant / persistent SBUF tensors ----
        xT_sb = const.tile([P, NSB, NKO, 512], bf16, name="xT_sb")
        wq_sb = const.tile([P, NKO, P], bf16, name="wq_sb")
        wk_sb = const.tile([P, NKO, P], bf16, name="wk_sb")
        wv_sb = const.tile([P, NKO, P], bf16, name="wv_sb")
        wo_sb = const.tile([P, NKO, E], bf16, name="wo_sb")
        bo_sb = const.tile([P, NKO], f32, name="bo_sb")
        masks_sb = const.tile([P, 2048], bf16, name="masks_sb")
        qT_sb = const.tile([P, SG], bf16, name="qT_sb")
        kT_sb = const.tile([P, SG], bf16, name="kT_sb")
        # per seq-tile: [ones | v_h0(64) | ones | v_h1(64)] -- the leading ones
        # column makes the softmax denominator land on PSUM partition 0
        v_aug = const.tile([P, NST, 130], bf16, name="v_aug")

        # interleave weight loads with the first x chunks so the first
        # projection blocks start as early as possible
        def load_x(sb):
            nc.sync.dma_start(xT_sb[:, sb], xT[:, sb])
        nc.sync.dma_start(wq_sb, wq.rearrange("(ko p) m -> p ko m", p=P))
        load_x(0)
        nc.sync.dma_start(wk_sb, wk.rearrange("(ko p) m -> p ko m", p=P))
        load_x(1)
        nc.sync.dma_start(wv_sb, wv.rearrange("(ko p) m -> p ko m", p=P))
        for sb in range(2, NSB):
            load_x(sb)
        nc.sync.dma_start(masks_sb, masks[:])
        nc.sync.dma_start(bo_sb, bo[:])
        nc.sync.dma_start(wo_sb, wo.rearrange("(ko p) m -> p ko m", p=P))

        with tc.tile_pool(name="psBig", bufs=2, space="PSUM") as psBig, \
             tc.tile_pool(name="psSmall", bufs=4, space="PSUM") as psSmall, \
             tc.tile_pool(name="sb_att", bufs=3) as sba:
            nc.any.memset(v_aug[:, :, 0:1], 1.0)
            nc.any.memset(v_aug[:, :, 65:66], 1.0)

            def proj_block(w_sb, dst, sb):
                ps = psBig.tile([P, 2, 512], f32, tag="big", name="ps_proj")
                for ko in range(NKO):
                    nc.tensor.matmul(
                        ps[:, 0, :],
                        w_sb[:, ko, :],
                        xT_sb[:, sb, ko, :],
                        start=(ko == 0),
                        stop=(ko == NKO - 1),
                    )
                nc.vector.tensor_copy(out=dst[:, sb * 512:(sb + 1) * 512], in_=ps[:, 0, :])

            def v_block(st):
                ps = psSmall.tile([P, P], f32, tag="small", name="ps_vproj")
                co = (st % 4) * P
                for ko in range(NKO):
                    nc.tensor.matmul(
                        ps,
                        xT_sb[:, st // 4, ko, co:co + P],
                        wv_sb[:, ko, :],
                        start=(ko == 0),
                        stop=(ko == NKO - 1),
                    )
                nc.vector.tensor_copy(
                    out=v_aug[:, st, 0:130].rearrange("p (h x) -> p h x", x=65)[:, :, 1:65],
                    in_=ps.rearrange("p (h x) -> p h x", x=64),
                )

            def attn_unit(b, qb):
                # software-pipelined: sc/exp of k-tile i+1 emitted before the
                # PV of k-tile i, so PE never stalls on ACT. Score tiles are
                # hl-paired [128(keys), 2(heads), 512(q)].
                numer = [
                    psSmall.tile([65, 512], f32, tag="small", name="ps_nm_t")
                    for _ in range(2)
                ]
                nkt = 4 * qb + 4
                q0 = S * b + qb * 512
                exs = {}

                def emit_sc_exp(kt):
                    sc = psBig.tile([P, 2, 512], f32, tag="big", name="ps_sc_t")
                    ex = sba.tile([P, 2, 512], bf16, tag="exp", name="sb_ex_t")
                    for hl in range(2):
                        nc.tensor.matmul(
                            sc[:, hl, :],
                            kT_sb[64 * hl:64 * hl + 64,
                                  S * b + kt * P:S * b + (kt + 1) * P],
                            qT_sb[64 * hl:64 * hl + 64, q0:q0 + 512],
                            start=True,
                            stop=True,
                        )
                    dj = kt - 4 * qb
                    qv = 128 * dj if dj > 0 else 0
                    # exp only the q-range PV will read (strided over hl)
                    nc.scalar.activation(
                        ex[:, :, qv:512], sc[:, :, qv:512], Exp, scale=0.125
                    )
                    if dj >= 0:
                        # only the [128,128] diagonal square needs masking;
                        # leading q-columns are skipped by PV entirely
                        for hl in range(2):
                            nc.vector.tensor_mul(
                                out=ex[:, hl, 128 * dj:128 * dj + 128],
                                in0=ex[:, hl, 128 * dj:128 * dj + 128],
                                in1=masks_sb[:, 0:P],
                            )
                    exs[kt] = ex

                def emit_pv(kt):
                    dj = kt - 4 * qb
                    qv = 128 * dj if dj > 0 else 0
                    ex = exs.pop(kt)
                    for hl in range(2):
                        nc.tensor.matmul(
                            numer[hl][:, qv:512],
                            v_aug[:, 16 * b + kt, 65 * hl:65 * hl + 65],
                            ex[:, hl, qv:512],
                            start=(kt == 0),
                            stop=(kt == nkt - 1),
                        )

                emit_sc_exp(0)
                for kt in range(1, nkt):
                    emit_sc_exp(kt)
                    emit_pv(kt - 1)
                emit_pv(nkt - 1)

                dst = a2a_in0 if b == 0 else a2a_in1
                slot = 4 * b + qb
                for hl in range(2):
                    recip = sba.tile([1, 512], f32, tag="recip", name="sb_rc_t")
                    nc.vector.reciprocal_approx_fast(recip, numer[hl][0:1, :])
                    rb = sba.tile([65, 512], f32, tag="rbcast", name="sb_rb_t")
                    nc.gpsimd.partition_broadcast(rb, recip)
                    attn = sba.tile([65, 512], bf16, tag="attn", name="sb_at_t")
                    nc.vector.tensor_mul(out=attn, in0=numer[hl][:, :], in1=rb)
                    nc.sync.dma_start(
                        dst[slot, 64 * hl:64 * hl + 64, :], attn[1:65, :]
                    )

            def a2a(g):
                src = a2a_in0 if g == 0 else a2a_in1
                nc.gpsimd.collective_compute(
                    "AllToAll",
                    mybir.AluOpType.bypass,
                    replica_groups=[list(range(8))],
                    ins=[src[:].opt()],
                    outs=[a2a_out[g].opt()],
                )

            # batch-0 inputs first
            for sb in range(4):
                proj_block(wq_sb, qT_sb, sb)
                proj_block(wk_sb, kT_sb, sb)
            for st in range(16):
                v_block(st)

            # batch-1 projection thunks, grouped by seq-block (unit (1,j)
            # only needs block 4+j), spread across BOTH batches' attention
            # units so the ACT-bound batch-1 units keep the PE fed
            a1 = []
            for sb in range(4, 8):
                a1.append(lambda sb=sb: proj_block(wq_sb, qT_sb, sb))
                a1.append(lambda sb=sb: proj_block(wk_sb, kT_sb, sb))
                for st in range(4 * sb, 4 * sb + 4):
                    a1.append(lambda st=st: v_block(st))

            def take(n):
                nonlocal a1
                batch, a1 = a1[:n], a1[n:]
                for thunk in batch:
                    thunk()

            for qb in range(QB):
                attn_unit(0, qb)
                take(3)            # groups sb4, sb5 during batch 0
            a2a(0)  # exchange batch-0 slots under batch-1 compute
            for qb in range(QB):
                attn_unit(1, qb)
                if qb < 2:
                    take(3)        # group sb6 during units (1,0),(1,1)
                elif qb == 2:
                    take(6)        # group sb7 before unit (1,3)
            a2a(1)

            # ---- output projection; this core's A2A half selected by a
            # pid//4-offset DMA ----
            pid = nc.sync.partition_id()
            g = pid // 4
            attn_all = const.tile([P, 8, 512], bf16, name="attn_all")
            for ci in range(8):
                nc.sync.dma_start(
                    attn_all[:, ci, :],
                    a2a_out[bass_ds(g, 1), ci].rearrange("o p f -> (o p) f"),
                )
            out_sb = const.tile([P, NKO, 512], bf16, name="out_sb")
            outT_r = outT.rearrange("(mo p) f -> p mo f", p=P)
            for mo in range(NKO):
                ps = psBig.tile([P, 2, 512], f32, tag="big", name="ps_out")
                for ci in range(8):
                    nc.tensor.matmul(
                        ps[:, 0, :],
                        wo_sb[:, ci, mo * P:(mo + 1) * P],
                        attn_all[:, ci, :],
                        start=(ci == 0),
                        stop=(ci == 7),
                    )
                nc.scalar.activation(
                    out_sb[:, mo, :], ps[:, 0, :], Identity,
                    bias=bo_sb[:, mo:mo + 1], scale=1.0,
                )
                nc.sync.dma_start(outT_r[:, mo:mo + 1, :], out_sb[:, mo:mo + 1, :])

    nc.compile()
    _built = nc
    return nc


def _host_masks():
    p = np.arange(P)[:, None]
    f = np.arange(512)[None, :]
    m = np.zeros((P, 4, 512), np.float32)
    for r in range(4):
        m[:, r, :] = (f >= P * r + p).astype(np.float32)
    return np.ascontiguousarray(m.reshape(P, 2048)).astype(BF16)


def kernel(**inputs):
    global LAST_RESULTS
    from concourse import bass_utils

    x = np.asarray(inputs["x"], np.float32)
    W_q = np.asarray(inputs["W_q"], np.float32)
    W_k = np.asarray(inputs["W_k"], np.float32)
    W_v = np.asarray(inputs["W_v"], np.float32)
    W_o = np.asarray(inputs["W_o"], np.float32)
    b_o = np.asarray(inputs["b_o"], np.float32)

    nc = _build()

    xT_all = np.concatenate([x[0].T, x[1].T], axis=1)   # [E, SG]
    # -> [partition, seq-block, ko, 512] (8KB-contiguous chunk lines)
    xT_all = np.ascontiguousarray(
        xT_all.reshape(NKO, P, NSB, 512).transpose(1, 2, 0, 3)
    ).astype(BF16)
    wo_b = np.ascontiguousarray(W_o).astype(BF16)
    bo_t = np.ascontiguousarray(b_o.reshape(NKO, P).T).astype(np.float32)
    masks = _host_masks()

    in_maps = []
    for c in range(8):
        sl = slice(P * c, P * (c + 1))
        in_maps.append({
            "xT": xT_all,
            "wq": np.ascontiguousarray(W_q[:, sl]).astype(BF16),
            "wk": np.ascontiguousarray(W_k[:, sl]).astype(BF16),
            "wv": np.ascontiguousarray(W_v[:, sl]).astype(BF16),
            "wo": wo_b,
            "bo": bo_t,
            "masks": masks,
        })

    res = bass_utils.run_bass_kernel_spmd(nc, in_maps, core_ids=list(range(8)))
    LAST_RESULTS = res

    out = np.empty((B, S, E), np.float32)
    for c in range(8):
        b, qb = c // 4, c % 4
        out[b, 512 * qb:512 * (qb + 1), :] = np.asarray(
            res.results[c]["outT"], np.float32
        ).T
    return out.astype(np.float32)


# revision 39
# speedup vs baseline: 1.1857x; 1.1857x over previous
"""Masked multi-head attention (B=2, S=2048, E=1024, H=16, D=64) on 8 TRN2 cores.

Sharding: each core owns 2 heads (of 16) for BOTH batches.
  - QKV projections computed per-core for its 2 heads (transposed layouts);
    batch-1 projection blocks are interleaved between batch-0 attention units
    so the TensorEngine stays busy while ScalarE runs exp.
  - Attention: flash-style with transposed scores (scoresT[k, q] tiles),
    software-pipelined at k-tile granularity: the score matmuls of k-tile
    i+1 are emitted BEFORE the PV matmuls of k-tile i, so the PE streams
    the next scores while ScalarE runs exp on the previous tile (PSUM score
    tiles are hl-paired [128, 2, 512] so one pool buf covers both heads).
    Unsafe softmax (no max subtraction -- scores are ~N(0,1)); denominator
    accumulated via a leading ones-column prepended to V in the PV matmul;
    causal mask applied only on the [128,128] diagonal square, PV matmuls
    skip fully-masked leading q-columns of diagonal k-tiles.
  - Exchange head-parallel -> slot-parallel via TWO AllToAlls: slots 0-3
    (batch 0) exchanged right after batch-0 attention, hidden under batch-1
    compute; slots 4-7 at the end (only this one is exposed). AllToAll
    moves 8x less data than per-slot AllGathers and avoids CC serialization.
  - Output projection row-parallel over the received heads (each core picks
    its A2A output half via a pid//4-offset DMA), bias fused via ScalarE;
    each core emits a transposed [1024, 512] bf16 slice; host transposes,
    stacks and upcasts.

Compute dtype bf16 (fp32 PSUM accumulation). Exp is trimmed to the valid
q-range of diagonal k-tiles (ScalarE is the near-critical engine during
attention). HW exec time ~230us (from 256.6us baseline); rel-l2 error vs
the fp32 reference ~5.3e-3.
"""

import numpy as np
import ml_dtypes

BF16 = ml_dtypes.bfloat16

B, S, E, H, D = 2, 2048, 1024, 16, 64
P = 128
SG = B * S          # 4096 global sequence length (batch-major)
NKO = E // P        # 8 contraction tiles over E
NST = SG // P       # 32 seq tiles of 128
NSB = SG // 512     # 8 seq blocks of 512
QB = S // 512       # 4 q-blocks per batch

_built = None
LAST_RESULTS = None


def _build():
    global _built
    if _built is not None:
        return _built

    import concourse.bacc as bacc
    import concourse.mybir as mybir
    import concourse.tile as tile
    from concourse.bass import ds as bass_ds

    f32 = mybir.dt.float32
    bf16 = mybir.dt.bfloat16
    Exp = mybir.ActivationFunctionType.Exp
    Identity = mybir.ActivationFunctionType.Identity

    nc = bacc.Bacc("TRN2", target_bir_lowering=False, debug=False, num_devices=8)

    # x pre-rearranged on host to [partition, seq-block, ko, 512] so each
    # chunk DMA moves 8KB-contiguous lines per partition on both sides
    xT = nc.declare_dram_parameter("xT", [P, NSB, NKO, 512], bf16, isOutput=False)
    wq = nc.declare_dram_parameter("wq", [E, P], bf16, isOutput=False)
    wk = nc.declare_dram_parameter("wk", [E, P], bf16, isOutput=False)
    wv = nc.declare_dram_parameter("wv", [E, P], bf16, isOutput=False)
    wo = nc.declare_dram_parameter("wo", [E, E], bf16, isOutput=False)
    bo = nc.declare_dram_parameter("bo", [P, NKO], f32, isOutput=False)
    masks = nc.declare_dram_parameter("masks", [P, 2048], bf16, isOutput=False)
    outT = nc.declare_dram_parameter("outT", [E, 512], bf16, isOutput=True)

    # AllToAll exchange, split by batch so the first one hides under compute:
    #   a2a_in0 chunk j (j<4) = my heads' attention for slot j   (batch 0)
    #   a2a_in1 chunk j (j>=4) = my heads' attention for slot j  (batch 1)
    # (unwritten chunks carry junk to cores that ignore them)
    # a2a_out[g] = output of A2A g; core c reads half g = c//4: chunk r there
    # holds rank r's heads for core c's own slot.
    a2a_in0 = nc.dram_tensor("a2a_in0", [8, P, 512], bf16)
    a2a_in1 = nc.dram_tensor("a2a_in1", [8, P, 512], bf16)
    a2a_out = nc.dram_tensor("a2a_out", [2, 8, P, 512], bf16)

    with tile.TileContext(nc) as tc, \
         tc.tile_pool(name="const", bufs=1) as const:
        # ---- constant / persistent SBUF tensors ----
        xT_sb = const.tile([P, NSB, NKO, 512], bf16, name="xT_sb")
        wq_sb = const.tile([P, NKO, P], bf16, name="wq_sb")
        wk_sb = const.tile([P, NKO, P], bf16, name="wk_sb")
        wv_sb = const.tile([P, NKO, P], bf16, name="wv_sb")
        wo_sb = const.tile([P, NKO, E], bf16, name="wo_sb")
        bo_sb = const.tile([P, NKO], f32, name="bo_sb")
        masks_sb = const.tile([P, 2048], bf16, name="masks_sb")
        qT_sb = const.tile([P, SG], bf16, name="qT_sb")
        kT_sb = const.tile([P, SG], bf16, name="kT_sb")
        # per seq-tile: [ones | v_h0(64) | ones | v_h1(64)] -- the leading ones
        # column makes the softmax denominator land on PSUM partition 0
        v_aug = const.tile([P, NST, 130], bf16, name="v_aug")

        # interleave weight loads with the first x chunks so the first
        # projection blocks start as early as possible
        def load_x(sb):
            nc.sync.dma_start(xT_sb[:, sb], xT[:, sb])
        nc.sync.dma_start(wq_sb, wq.rearrange("(ko p) m -> p ko m", p=P))
        load_x(0)
        nc.sync.dma_start(wk_sb, wk.rearrange("(ko p) m -> p ko m", p=P))
        load_x(1)
        nc.sync.dma_start(wv_sb, wv.rearrange("(ko p) m -> p ko m", p=P))
        for sb in range(2, NSB):
            load_x(sb)
        nc.sync.dma_start(masks_sb, masks[:])
        nc.sync.dma_start(bo_sb, bo[:])
        nc.sync.dma_start(wo_sb, wo.rearrange("(ko p) m -> p ko m", p=P))

        with tc.tile_pool(name="psBig", bufs=2, space="PSUM") as psBig, \
             tc.tile_pool(name="psSmall", bufs=4, space="PSUM") as psSmall, \
             tc.tile_pool(name="sb_att", bufs=3) as sba:
            nc.any.memset(v_aug[:, :, 0:1], 1.0)
            nc.any.memset(v_aug[:, :, 65:66], 1.0)
            pid = nc.sync.partition_id()

            def proj_block(w_sb, dst, sb):
                ps = psBig.tile([P, 2, 512], f32, tag="big", name="ps_proj")
                for ko in range(NKO):
                    nc.tensor.matmul(
                        ps[:, 0, :],
                        w_sb[:, ko, :],
                        xT_sb[:, sb, ko, :],
                        start=(ko == 0),
                        stop=(ko == NKO - 1),
                    )
                nc.vector.tensor_copy(out=dst[:, sb * 512:(sb + 1) * 512], in_=ps[:, 0, :])

            def v_block(st):
                ps = psSmall.tile([P, P], f32, tag="small", name="ps_vproj")
                co = (st % 4) * P
                for ko in range(NKO):
                    nc.tensor.matmul(
                        ps,
                        xT_sb[:, st // 4, ko, co:co + P],
                        wv_sb[:, ko, :],
                        start=(ko == 0),
                        stop=(ko == NKO - 1),
                    )
                nc.vector.tensor_copy(
                    out=v_aug[:, st, 0:130].rearrange("p (h x) -> p h x", x=65)[:, :, 1:65],
                    in_=ps.rearrange("p (h x) -> p h x", x=64),
                )

            def attn_unit(b, qb):
                # software-pipelined: sc/exp of k-tile i+1 emitted before the
                # PV of k-tile i, so PE never stalls on ACT. Score tiles are
                # hl-paired [128(keys), 2(heads), 512(q)].
                numer = [
                    psSmall.tile([65, 512], f32, tag="small", name="ps_nm_t")
                    for _ in range(2)
                ]
                nkt = 4 * qb + 4
                q0 = S * b + qb * 512
                exs = {}

                def emit_sc_exp(kt):
                    sc = psBig.tile([P, 2, 512], f32, tag="big", name="ps_sc_t")
                    ex = sba.tile([P, 2, 512], bf16, tag="exp", name="sb_ex_t")
                    for hl in range(2):
                        nc.tensor.matmul(
                            sc[:, hl, :],
                            kT_sb[64 * hl:64 * hl + 64,
                                  S * b + kt * P:S * b + (kt + 1) * P],
                            qT_sb[64 * hl:64 * hl + 64, q0:q0 + 512],
                            start=True,
                            stop=True,
                        )
                    dj = kt - 4 * qb
                    qv = 128 * dj if dj > 0 else 0
                    # exp only the q-range PV will read (strided over hl)
                    nc.scalar.activation(
                        ex[:, :, qv:512], sc[:, :, qv:512], Exp, scale=0.125
                    )
                    if dj >= 0:
                        # only the [128,128] diagonal square needs masking;
                        # leading q-columns are skipped by PV entirely
                        for hl in range(2):
                            nc.vector.tensor_mul(
                                out=ex[:, hl, 128 * dj:128 * dj + 128],
                                in0=ex[:, hl, 128 * dj:128 * dj + 128],
                                in1=masks_sb[:, 0:P],
                            )
                    exs[kt] = ex

                def emit_pv(kt):
                    dj = kt - 4 * qb
                    qv = 128 * dj if dj > 0 else 0
                    ex = exs.pop(kt)
                    for hl in range(2):
                        nc.tensor.matmul(
                            numer[hl][:, qv:512],
                            v_aug[:, 16 * b + kt, 65 * hl:65 * hl + 65],
                            ex[:, hl, qv:512],
                            start=(kt == 0),
                            stop=(kt == nkt - 1),
                        )

                emit_sc_exp(0)
                for kt in range(1, nkt):
                    emit_sc_exp(kt)
                    emit_pv(kt - 1)
                emit_pv(nkt - 1)

                dst = a2a_in0 if b == 0 else a2a_in1
                slot = 4 * b + qb
                for hl in range(2):
                    recip = sba.tile([1, 512], f32, tag="recip", name="sb_rc_t")
                    nc.vector.reciprocal_approx_fast(recip, numer[hl][0:1, :])
                    rb = sba.tile([65, 512], f32, tag="rbcast", name="sb_rb_t")
                    nc.gpsimd.partition_broadcast(rb, recip)
                    attn = sba.tile([65, 512], bf16, tag="attn", name="sb_at_t")
                    nc.vector.tensor_mul(out=attn, in0=numer[hl][:, :], in1=rb)
                    nc.sync.dma_start(
                        dst[slot, 64 * hl:64 * hl + 64, :], attn[1:65, :]
                    )

            def a2a(g):
                src = a2a_in0 if g == 0 else a2a_in1
                nc.gpsimd.collective_compute(
                    "AllToAll",
                    mybir.AluOpType.bypass,
                    replica_groups=[list(range(8))],
                    ins=[src[:].opt()],
                    outs=[a2a_out[g].opt()],
                )

            # batch-0 inputs first
            for sb in range(4):
                proj_block(wq_sb, qT_sb, sb)
                proj_block(wk_sb, kT_sb, sb)
            for st in range(16):
                v_block(st)

            # batch-1 projection thunks, grouped by seq-block (unit (1,j)
            # only needs block 4+j), spread across BOTH batches' attention
            # units so the ACT-bound batch-1 units keep the PE fed
            a1 = []
            for sb in range(4, 8):
                a1.append(lambda sb=sb: proj_block(wq_sb, qT_sb, sb))
                a1.append(lambda sb=sb: proj_block(wk_sb, kT_sb, sb))
                for st in range(4 * sb, 4 * sb + 4):
                    a1.append(lambda st=st: v_block(st))

            def take(n):
                nonlocal a1
                batch, a1 = a1[:n], a1[n:]
                for thunk in batch:
                    thunk()

            for qb in range(QB):
                attn_unit(0, qb)
                take(3)            # groups sb4, sb5 during batch 0
            a2a(0)  # exchange batch-0 slots under batch-1 compute
            for qb in range(QB):
                attn_unit(1, qb)
                if qb < 2:
                    take(3)        # group sb6 during units (1,0),(1,1)
                elif qb == 2:
                    take(6)        # group sb7 before unit (1,3)
            a2a(1)

            # ---- output projection; this core's A2A half selected by a
            # pid//4-offset DMA ----
            g = pid // 4
            attn_all = const.tile([P, 8, 512], bf16, name="attn_all")
            for ci in range(8):
                nc.sync.dma_start(
                    attn_all[:, ci, :],
                    a2a_out[bass_ds(g, 1), ci].rearrange("o p f -> (o p) f"),
                )
            out_sb = const.tile([P, NKO, 512], bf16, name="out_sb")
            outT_r = outT.rearrange("(mo p) f -> p mo f", p=P)
            for mo in range(NKO):
                ps = psBig.tile([P, 2, 512], f32, tag="big", name="ps_out")
                for ci in range(8):
                    nc.tensor.matmul(
                        ps[:, 0, :],
                        wo_sb[:, ci, mo * P:(mo + 1) * P],
                        attn_all[:, ci, :],
                        start=(ci == 0),
                        stop=(ci == 7),
                    )
                nc.scalar.activation(
                    out_sb[:, mo, :], ps[:, 0, :], Identity,
                    bias=bo_sb[:, mo:mo + 1], scale=1.0,
                )
                nc.sync.dma_start(outT_r[:, mo:mo + 1, :], out_sb[:, mo:mo + 1, :])

    nc.compile()
    _built = nc
    return nc


def _host_masks():
    p = np.arange(P)[:, None]
    f = np.arange(512)[None, :]
    m = np.zeros((P, 4, 512), np.float32)
    for r in range(4):
        m[:, r, :] = (f >= P * r + p).astype(np.float32)
    return np.ascontiguousarray(m.reshape(P, 2048)).astype(BF16)


def kernel(**inputs):
    global LAST_RESULTS
    from concourse import bass_utils

    x = np.asarray(inputs["x"], np.float32)
    W_q = np.asarray(inputs["W_q"], np.float32)
    W_k = np.asarray(inputs["W_k"], np.float32)
    W_v = np.asarray(inputs["W_v"], np.float32)
    W_o = np.asarray(inputs["W_o"], np.float32)
    b_o = np.asarray(inputs["b_o"], np.float32)

    nc = _build()

    xT_all = np.concatenate([x[0].T, x[1].T], axis=1)   # [E, SG]
    # -> [partition, seq-block, ko, 512] (8KB-contiguous chunk lines)
    xT_all = np.ascontiguousarray(
        xT_all.reshape(NKO, P, NSB, 512).transpose(1, 2, 0, 3)
    ).astype(BF16)
    wo_b = np.ascontiguousarray(W_o).astype(BF16)
    bo_t = np.ascontiguousarray(b_o.reshape(NKO, P).T).astype(np.float32)
    masks = _host_masks()

    in_maps = []
    for c in range(8):
        sl = slice(P * c, P * (c + 1))
        in_maps.append({
            "xT": xT_all,
            "wq": np.ascontiguousarray(W_q[:, sl]).astype(BF16),
            "wk": np.ascontiguousarray(W_k[:, sl]).astype(BF16),
            "wv": np.ascontiguousarray(W_v[:, sl]).astype(BF16),
            "wo": wo_b,
            "bo": bo_t,
            "masks": masks,
        })

    res = bass_utils.run_bass_kernel_spmd(nc, in_maps, core_ids=list(range(8)))
    LAST_RESULTS = res

    out = np.empty((B, S, E), np.float32)
    for c in range(8):
        b, qb = c // 4, c % 4
        out[b, 512 * qb:512 * (qb + 1), :] = np.asarray(
            res.results[c]["outT"], np.float32
        ).T
    return out.astype(np.float32)


# revision 41
# speedup vs baseline: 1.1866x; 1.0008x over previous
"""Masked multi-head attention (B=2, S=2048, E=1024, H=16, D=64) on 8 TRN2 cores.

Sharding: each core owns 2 heads (of 16) for BOTH batches.
  - QKV projections computed per-core for its 2 heads (transposed layouts);
    batch-1 projection blocks are interleaved between batch-0 attention units
    so the TensorEngine stays busy while ScalarE runs exp.
  - Attention: flash-style with transposed scores (scoresT[k, q] tiles),
    software-pipelined at k-tile granularity: the score matmuls of k-tile
    i+1 are emitted BEFORE the PV matmuls of k-tile i, so the PE streams
    the next scores while ScalarE runs exp on the previous tile (PSUM score
    tiles are hl-paired [128, 2, 512] so one pool buf covers both heads).
    Unsafe softmax (no max subtraction -- scores are ~N(0,1)); denominator
    accumulated via a leading ones-column prepended to V in the PV matmul;
    causal mask applied only on the [128,128] diagonal square, PV matmuls
    skip fully-masked leading q-columns of diagonal k-tiles.
  - Exchange head-parallel -> slot-parallel via TWO AllToAlls: slots 0-3
    (batch 0) exchanged right after batch-0 attention, hidden under batch-1
    compute; slots 4-7 at the end (only this one is exposed). AllToAll
    moves 8x less data than per-slot AllGathers and avoids CC serialization.
  - Output projection row-parallel over the received heads (each core picks
    its A2A output half via a pid//4-offset DMA), bias fused via ScalarE;
    each core emits a transposed [1024, 512] bf16 slice; host transposes,
    stacks and upcasts.

Compute dtype bf16 (fp32 PSUM accumulation). Exp is trimmed to the valid
q-range of diagonal k-tiles (ScalarE is the near-critical engine during
attention). Batch-1 projection thunks are spread across BOTH batches'
attention units (unit (1,j) only needs seq-block 4+j, so its inputs can be
produced as late as unit (1,j-1)), and x is staged seq-block-major so every
chunk DMA moves 8KB-contiguous lines. Attention compute ends ~165us; HW
exec time ~232us median (from 256.6us baseline), the tail dominated by the
final AllToAll's rendezvous latency (noisy, 13-35us). Rel-l2 error vs the
fp32 reference ~5.3e-3.
"""

import numpy as np
import ml_dtypes

BF16 = ml_dtypes.bfloat16

B, S, E, H, D = 2, 2048, 1024, 16, 64
P = 128
SG = B * S          # 4096 global sequence length (batch-major)
NKO = E // P        # 8 contraction tiles over E
NST = SG // P       # 32 seq tiles of 128
NSB = SG // 512     # 8 seq blocks of 512
QB = S // 512       # 4 q-blocks per batch

_built = None
LAST_RESULTS = None


def _build():
    global _built
    if _built is not None:
        return _built

    import concourse.bacc as bacc
    import concourse.mybir as mybir
    import concourse.tile as tile
    from concourse.bass import ds as bass_ds

    f32 = mybir.dt.float32
    bf16 = mybir.dt.bfloat16
    Exp = mybir.ActivationFunctionType.Exp
    Identity = mybir.ActivationFunctionType.Identity

    nc = bacc.Bacc("TRN2", target_bir_lowering=False, debug=False, num_devices=8)

    # x pre-rearranged on host to [partition, seq-block, ko, 512] so each
    # chunk DMA moves 8KB-contiguous lines per partition on both sides
    xT = nc.declare_dram_parameter("xT", [P, NSB, NKO, 512], bf16, isOutput=False)
    wq = nc.declare_dram_parameter("wq", [E, P], bf16, isOutput=False)
    wk = nc.declare_dram_parameter("wk", [E, P], bf16, isOutput=False)
    wv = nc.declare_dram_parameter("wv", [E, P], bf16, isOutput=False)
    wo = nc.declare_dram_parameter("wo", [E, E], bf16, isOutput=False)
    bo = nc.declare_dram_parameter("bo", [P, NKO], f32, isOutput=False)
    masks = nc.declare_dram_parameter("masks", [P, 2048], bf16, isOutput=False)
    outT = nc.declare_dram_parameter("outT", [E, 512], bf16, isOutput=True)

    # AllToAll exchange, split by batch so the first one hides under compute:
    #   a2a_in0 chunk j (j<4) = my heads' attention for slot j   (batch 0)
    #   a2a_in1 chunk j (j>=4) = my heads' attention for slot j  (batch 1)
    # (unwritten chunks carry junk to cores that ignore them)
    # a2a_out[g] = output of A2A g; core c reads half g = c//4: chunk r there
    # holds rank r's heads for core c's own slot.
    a2a_in0 = nc.dram_tensor("a2a_in0", [8, P, 512], bf16)
    a2a_in1 = nc.dram_tensor("a2a_in1", [8, P, 512], bf16)
    a2a_out = nc.dram_tensor("a2a_out", [2, 8, P, 512], bf16)

    with tile.TileContext(nc) as tc, \
         tc.tile_pool(name="const", bufs=1) as const:
        # ---- constant / persistent SBUF tensors ----
        xT_sb = const.tile([P, NSB, NKO, 512], bf16, name="xT_sb")
        wq_sb = const.tile([P, NKO, P], bf16, name="wq_sb")
        wk_sb = const.tile([P, NKO, P], bf16, name="wk_sb")
        wv_sb = const.tile([P, NKO, P], bf16, name="wv_sb")
        wo_sb = const.tile([P, NKO, E], bf16, name="wo_sb")
        bo_sb = const.tile([P, NKO], f32, name="bo_sb")
        masks_sb = const.tile([P, 2048], bf16, name="masks_sb")
        qT_sb = const.tile([P, SG], bf16, name="qT_sb")
        kT_sb = const.tile([P, SG], bf16, name="kT_sb")
        # per seq-tile: [ones | v_h0(64) | ones | v_h1(64)] -- the leading ones
        # column makes the softmax denominator land on PSUM partition 0
        v_aug = const.tile([P, NST, 130], bf16, name="v_aug")

        # interleave weight loads with the first x chunks so the first
        # projection blocks start as early as possible
        def load_x(sb):
            nc.sync.dma_start(xT_sb[:, sb], xT[:, sb])
        # weights on the gpsimd DMA queue, x chunks on the sync queue: the
        # two queues run in parallel so the first projection starts sooner
        # (both are drained long before the first collective dispatch)
        nc.gpsimd.dma_start(wq_sb, wq.rearrange("(ko p) m -> p ko m", p=P))
        nc.gpsimd.dma_start(wk_sb, wk.rearrange("(ko p) m -> p ko m", p=P))
        nc.gpsimd.dma_start(wv_sb, wv.rearrange("(ko p) m -> p ko m", p=P))
        for sb in range(NSB):
            load_x(sb)
        nc.gpsimd.dma_start(masks_sb, masks[:])
        nc.gpsimd.dma_start(bo_sb, bo[:])
        nc.gpsimd.dma_start(wo_sb, wo.rearrange("(ko p) m -> p ko m", p=P))

        with tc.tile_pool(name="psBig", bufs=2, space="PSUM") as psBig, \
             tc.tile_pool(name="psSmall", bufs=4, space="PSUM") as psSmall, \
             tc.tile_pool(name="sb_att", bufs=3) as sba:
            nc.any.memset(v_aug[:, :, 0:1], 1.0)
            nc.any.memset(v_aug[:, :, 65:66], 1.0)
            pid = nc.sync.partition_id()

            def proj_block(w_sb, dst, sb):
                ps = psBig.tile([P, 2, 512], f32, tag="big", name="ps_proj")
                for ko in range(NKO):
                    nc.tensor.matmul(
                        ps[:, 0, :],
                        w_sb[:, ko, :],
                        xT_sb[:, sb, ko, :],
                        start=(ko == 0),
                        stop=(ko == NKO - 1),
                    )
                nc.vector.tensor_copy(out=dst[:, sb * 512:(sb + 1) * 512], in_=ps[:, 0, :])

            def v_block(st):
                ps = psSmall.tile([P, P], f32, tag="small", name="ps_vproj")
                co = (st % 4) * P
                for ko in range(NKO):
                    nc.tensor.matmul(
                        ps,
                        xT_sb[:, st // 4, ko, co:co + P],
                        wv_sb[:, ko, :],
                        start=(ko == 0),
                        stop=(ko == NKO - 1),
                    )
                nc.vector.tensor_copy(
                    out=v_aug[:, st, 0:130].rearrange("p (h x) -> p h x", x=65)[:, :, 1:65],
                    in_=ps.rearrange("p (h x) -> p h x", x=64),
                )

            def attn_unit(b, qb):
                # software-pipelined: sc/exp of k-tile i+1 emitted before the
                # PV of k-tile i, so PE never stalls on ACT. Score tiles are
                # hl-paired [128(keys), 2(heads), 512(q)].
                numer = [
                    psSmall.tile([65, 512], f32, tag="small", name="ps_nm_t")
                    for _ in range(2)
                ]
                nkt = 4 * qb + 4
                q0 = S * b + qb * 512
                exs = {}

                def emit_sc_exp(kt):
                    sc = psBig.tile([P, 2, 512], f32, tag="big", name="ps_sc_t")
                    ex = sba.tile([P, 2, 512], bf16, tag="exp", name="sb_ex_t")
                    for hl in range(2):
                        nc.tensor.matmul(
                            sc[:, hl, :],
                            kT_sb[64 * hl:64 * hl + 64,
                                  S * b + kt * P:S * b + (kt + 1) * P],
                            qT_sb[64 * hl:64 * hl + 64, q0:q0 + 512],
                            start=True,
                            stop=True,
                        )
                    dj = kt - 4 * qb
                    qv = 128 * dj if dj > 0 else 0
                    # exp only the q-range PV will read (strided over hl)
                    nc.scalar.activation(
                        ex[:, :, qv:512], sc[:, :, qv:512], Exp, scale=0.125
                    )
                    if dj >= 0:
                        # only the [128,128] diagonal square needs masking;
                        # leading q-columns are skipped by PV entirely
                        for hl in range(2):
                            nc.vector.tensor_mul(
                                out=ex[:, hl, 128 * dj:128 * dj + 128],
                                in0=ex[:, hl, 128 * dj:128 * dj + 128],
                                in1=masks_sb[:, 0:P],
                            )
                    exs[kt] = ex

                def emit_pv(kt):
                    dj = kt - 4 * qb
                    qv = 128 * dj if dj > 0 else 0
                    ex = exs.pop(kt)
                    for hl in range(2):
                        nc.tensor.matmul(
                            numer[hl][:, qv:512],
                            v_aug[:, 16 * b + kt, 65 * hl:65 * hl + 65],
                            ex[:, hl, qv:512],
                            start=(kt == 0),
                            stop=(kt == nkt - 1),
                        )

                emit_sc_exp(0)
                for kt in range(1, nkt):
                    emit_sc_exp(kt)
                    emit_pv(kt - 1)
                emit_pv(nkt - 1)

                dst = a2a_in0 if b == 0 else a2a_in1
                slot = 4 * b + qb
                for hl in range(2):
                    recip = sba.tile([1, 512], f32, tag="recip", name="sb_rc_t")
                    nc.vector.reciprocal_approx_fast(recip, numer[hl][0:1, :])
                    rb = sba.tile([65, 512], f32, tag="rbcast", name="sb_rb_t")
                    nc.gpsimd.partition_broadcast(rb, recip)
                    attn = sba.tile([65, 512], bf16, tag="attn", name="sb_at_t")
                    nc.vector.tensor_mul(out=attn, in0=numer[hl][:, :], in1=rb)
                    nc.sync.dma_start(
                        dst[slot, 64 * hl:64 * hl + 64, :], attn[1:65, :]
                    )

            def a2a(g):
                src = a2a_in0 if g == 0 else a2a_in1
                nc.gpsimd.collective_compute(
                    "AllToAll",
                    mybir.AluOpType.bypass,
                    replica_groups=[list(range(8))],
                    ins=[src[:].opt()],
                    outs=[a2a_out[g].opt()],
                )

            # batch-0 inputs first
            for sb in range(4):
                proj_block(wq_sb, qT_sb, sb)
                proj_block(wk_sb, kT_sb, sb)
            for st in range(16):
                v_block(st)

            # batch-1 projection thunks, grouped by seq-block (unit (1,j)
            # only needs block 4+j), spread across BOTH batches' attention
            # units so the ACT-bound batch-1 units keep the PE fed
            a1 = []
            for sb in range(4, 8):
                a1.append(lambda sb=sb: proj_block(wq_sb, qT_sb, sb))
                a1.append(lambda sb=sb: proj_block(wk_sb, kT_sb, sb))
                for st in range(4 * sb, 4 * sb + 4):
                    a1.append(lambda st=st: v_block(st))

            def take(n):
                nonlocal a1
                batch, a1 = a1[:n], a1[n:]
                for thunk in batch:
                    thunk()

            for qb in range(QB):
                attn_unit(0, qb)
                take(3)            # groups sb4, sb5 during batch 0
            a2a(0)  # exchange batch-0 slots under batch-1 compute
            for qb in range(QB):
                attn_unit(1, qb)
                if qb < 2:
                    take(3)        # group sb6 during units (1,0),(1,1)
                elif qb == 2:
                    take(6)        # group sb7 before unit (1,3)
            a2a(1)

            # ---- output projection; this core's A2A half selected by a
            # pid//4-offset DMA ----
            g = pid // 4
            attn_all = const.tile([P, 8, 512], bf16, name="attn_all")
            for ci in range(8):
                nc.sync.dma_start(
                    attn_all[:, ci, :],
                    a2a_out[bass_ds(g, 1), ci].rearrange("o p f -> (o p) f"),
                )
            out_sb = const.tile([P, NKO, 512], bf16, name="out_sb")
            outT_r = outT.rearrange("(mo p) f -> p mo f", p=P)
            for mo in range(NKO):
                ps = psBig.tile([P, 2, 512], f32, tag="big", name="ps_out")
                for ci in range(8):
                    nc.tensor.matmul(
                        ps[:, 0, :],
                        wo_sb[:, ci, mo * P:(mo + 1) * P],
                        attn_all[:, ci, :],
                        start=(ci == 0),
                        stop=(ci == 7),
                    )
                nc.scalar.activation(
                    out_sb[:, mo, :], ps[:, 0, :], Identity,
                    bias=bo_sb[:, mo:mo + 1], scale=1.0,
                )
                nc.sync.dma_start(outT_r[:, mo:mo + 1, :], out_sb[:, mo:mo + 1, :])

    nc.compile()
    _built = nc
    return nc


def _host_masks():
    p = np.arange(P)[:, None]
    f = np.arange(512)[None, :]
    m = np.zeros((P, 4, 512), np.float32)
    for r in range(4):
        m[:, r, :] = (f >= P * r + p).astype(np.float32)
    return np.ascontiguousarray(m.reshape(P, 2048)).astype(BF16)


def kernel(**inputs):
    global LAST_RESULTS
    from concourse import bass_utils

    x = np.asarray(inputs["x"], np.float32)
    W_q = np.asarray(inputs["W_q"], np.float32)
    W_k = np.asarray(inputs["W_k"], np.float32)
    W_v = np.asarray(inputs["W_v"], np.float32)
    W_o = np.asarray(inputs["W_o"], np.float32)
    b_o = np.asarray(inputs["b_o"], np.float32)

    nc = _build()

    xT_all = np.concatenate([x[0].T, x[1].T], axis=1)   # [E, SG]
    # -> [partition, seq-block, ko, 512] (8KB-contiguous chunk lines)
    xT_all = np.ascontiguousarray(
        xT_all.reshape(NKO, P, NSB, 512).transpose(1, 2, 0, 3)
    ).astype(BF16)
    wo_b = np.ascontiguousarray(W_o).astype(BF16)
    bo_t = np.ascontiguousarray(b_o.reshape(NKO, P).T).astype(np.float32)
    masks = _host_masks()

    in_maps = []
    for c in range(8):
        sl = slice(P * c, P * (c + 1))
        in_maps.append({
            "xT": xT_all,
            "wq": np.ascontiguousarray(W_q[:, sl]).astype(BF16),
            "wk": np.ascontiguousarray(W_k[:, sl]).astype(BF16),
            "wv": np.ascontiguousarray(W_v[:, sl]).astype(BF16),
            "wo": wo_b,
            "bo": bo_t,
            "masks": masks,
        })

    res = bass_utils.run_bass_kernel_spmd(nc, in_maps, core_ids=list(range(8)))
    LAST_RESULTS = res

    out = np.empty((B, S, E), np.float32)
    for c in range(8):
        b, qb = c // 4, c % 4
        out[b, 512 * qb:512 * (qb + 1), :] = np.asarray(
            res.results[c]["outT"], np.float32
        ).T
    return out.astype(np.float32)
